# revision 1
# baseline (speedup 1.0000x reference)
"""CorefHead Trainium2 kernel.

Reference computation (B=64, S=512, H=1024, HID=512):
  emb_a = span_mean(bert, offsets[:,0:2])   # [B,H]
  emb_b = span_mean(bert, offsets[:,2:4])   # [B,H]
  emb_p = bert[b, offsets[:,4]]             # [B,H]
  x = concat([emb_a, emb_b, emb_p], -1)     # [B,3H]
  h = leaky_relu(batchnorm_eval(x @ W1 + b1), 0.01)
  out = h @ W2 + b2                         # [B,3]

Strategy: pure data parallel, batch sharded 8 ways (8 batches/core),
memory-roofline oriented. All DMA queues share the same 16 SDMA engines
(~330 B/ns aggregate), so bytes shipped is the binding constraint:
  - Host ships only the rows each span/pron actually needs, packed
    partition-major (one contiguous DMA line per partition), with the 24
    mask columns (3 embeddings x 8 batches) prepended per 128-row chunk.
  - Long spans (>=T rows) ship fp8 e3m4 (quantization noise averages out
    over the span); short spans + pron rows ship bf16.
  - W1 rows for the two span embeddings ship fp8 e3m4 (prescaled x16,
    undone via the x scale); pron-embedding rows stay bf16 (pron x-values
    are ~12x larger, so they dominate W1's error sensitivity).
  - mm1 (PE): stationary = mask chunk [128, 24], moving = bert chunk
    [128, 512]x2 -> x accumulates in PSUM [24, 1024] over ALL chunks;
    chunks may cross batch boundaries freely.
  - A memset-fed dummy-matmul burst pre-warms the PE clock (HAM) while
    the first data DMAs are in flight.
  - mm2 = 24 matmuls (xT slice [128,8] stationary, W1 [128,512] moving)
    + BN bias via a K=1 ones-row matmul; W1 DMA is interleaved behind the
    bert stream across both HWDGE rings so mm2 is DMA-paced.
  - Tail: one DVE leaky op, 4 batched PE transposes, 4-chunk mm3, K=1 b2
    matmul. Host gathers per-core [3, 8] outputs and undoes the batch
    permutation.
"""

import numpy as np

B, S, H = 64, 512, 1024
HID = 512
EPS = 1e-5
NCORES = 8
BPC = B // NCORES  # batches per core
NMC = 3 * BPC      # mask columns: (embedding e, batch slot b) -> e*BPC + b
MW = NMC           # mask width (cols 0:NMC of each chunk)
CW = MW + H        # chunk width: mask + bert row

# Span rows >= T ship as fp8; shorter spans (and pron rows) ship bf16.
T_FP8 = 32
SPAN_DT = "e3"     # "e3", "e4", or "bf16"
# Ship W1 span-embedding rows (k < 2048) as fp8 e3m4, prescaled x16.
W1_SPLIT = True
W1_SCALE = 16.0
E3_P0 = 2          # chunks in the first fp8 DMA piece (PE start latency)
E3_PIECE = 2       # chunks per later fp8 DMA piece (rotating rings)
N_WARM = 12        # dummy matmuls to pre-warm the PE clock

TRACE = False
LAST_RESULT = None

_PROGRAM_CACHE: dict = {}


def _span_np_dt():
    import ml_dtypes
    return {
        "e3": ml_dtypes.float8_e3m4,
        "e4": ml_dtypes.float8_e4m3,
        "bf16": ml_dtypes.bfloat16,
    }[SPAN_DT]


def _build_program(nch8: int, nchb: int, bp: int):
    import concourse.bacc as bacc
    import concourse.tile as tile
    import concourse.mybir as mybir
    from concourse.bass import MemorySpace

    f32 = mybir.dt.float32
    bf = mybir.dt.bfloat16
    sdt = {"e3": mybir.dt.float8e3, "e4": mybir.dt.float8e4,
           "bf16": mybir.dt.bfloat16}[SPAN_DT]
    w8 = mybir.dt.float8e3

    nc = bacc.Bacc("TRN2", target_bir_lowering=False, debug=False,
                   num_devices=NCORES)

    KC = 3 * H // 128   # 24
    KS = 16             # fp8 W1 k-chunks (span embeddings)
    HC = H // 128       # 8

    e3_d = nc.dram_tensor("e3buf", [128, nch8, CW], sdt,
                          kind="ExternalInput").ap()
    bf_d = nc.dram_tensor("bfbuf", [bp, nchb, CW], bf,
                          kind="ExternalInput").ap()
    # hc-major W1: w1S[p, hc*2+e, n] (fp8 span rows), w1P[p, hc, n] (pron)
    if W1_SPLIT:
        w1s_d = nc.dram_tensor("w1S", [128, KS, HID], w8,
                               kind="ExternalInput").ap()
        w1p_d = nc.dram_tensor("w1P", [128, KC - KS, HID], bf,
                               kind="ExternalInput").ap()
    else:
        w1f_d = nc.dram_tensor("w1F", [128, KC, HID], bf,
                               kind="ExternalInput").ap()
    # cstA (bf16): [0:24]=ident24, [24:36]=w2P (4 chunks x 3)
    # cstB (bf16, 1 partition): [0:512]=bnb, [512:520]=ones, [520:523]=b2
    cstA_d = nc.dram_tensor("cstA", [128, 36], bf, kind="ExternalInput").ap()
    cstB_d = nc.dram_tensor("cstB", [1, 523], bf, kind="ExternalInput").ap()
    sfac_d = nc.dram_tensor("sfac", [NMC, 1], f32, kind="ExternalInput").ap()
    out_d = nc.dram_tensor("out", [3, BPC], f32, kind="ExternalOutput").ap()

    with tile.TileContext(nc) as tc:
        with (
            tc.tile_pool(name="data", bufs=1) as data,
            tc.tile_pool(name="work", bufs=1) as work,
            tc.tile_pool(name="psum_x", bufs=1, space=MemorySpace.PSUM) as psx,
            tc.tile_pool(name="psum_t", bufs=4, space=MemorySpace.PSUM) as pst,
            tc.tile_pool(name="psum_h", bufs=1, space=MemorySpace.PSUM) as psh,
        ):
            # --- PE pre-warm on memset junk (no DMA dependency); also
            # trigger the ACT engine's table loads for Identity/Lrelu off
            # the critical path.
            junk = work.tile([128, 512], bf, tag="junk")
            nc.gpsimd.memset(junk, 0.0)
            scr = psh.tile([24, 512], f32, tag="scratch")
            for _ in range(N_WARM):
                nc.tensor.matmul(scr, junk[:, 0:24], junk, start=True,
                                 stop=True)

            # --- DMAs: consts on the slow SWDGE ring (needed late, tiny);
            # bert pieces first on both HWDGE rings, W1 behind them.
            cstA_t = data.tile([128, 36], bf, tag="cstA")
            nc.gpsimd.dma_start(out=cstA_t, in_=cstA_d)
            cstB_t = data.tile([1, 523], bf, tag="cstB")
            nc.gpsimd.dma_start(out=cstB_t, in_=cstB_d)
            sfac_t = data.tile([NMC, 1], f32, tag="sfac")
            nc.gpsimd.dma_start(out=sfac_t, in_=sfac_d)

            ring_engines = [nc.sync, nc.scalar]
            ring_bytes = [0, 0]

            def ring(nbytes):
                r = int(np.argmin(ring_bytes))
                ring_bytes[r] += nbytes
                return ring_engines[r]

            bounds = [0, min(E3_P0, nch8)]
            while bounds[-1] < nch8:
                bounds.append(min(nch8, bounds[-1] + E3_PIECE))
            e3_ts = []
            for i in range(len(bounds) - 1):
                c0, c1 = bounds[i], bounds[i + 1]
                t = data.tile([128, c1 - c0, CW], sdt, tag=f"e3p{i}")
                ring((c1 - c0) * 128 * CW).dma_start(
                    out=t, in_=e3_d[:, c0:c1, :])
                e3_ts.append((c0, t))
            bf_t = data.tile([bp, nchb, CW], bf, tag="bfp")
            ring(nchb * bp * CW * 2).dma_start(out=bf_t, in_=bf_d)
            # W1 pieces in hc-halves so mm2's hc-major order matches arrival
            if W1_SPLIT:
                w1s_ts, w1p_ts = [], []
                for i in range(2):
                    t = data.tile([128, 8, HID], w8, tag=f"w1s{i}",
                                  name=f"w1s{i}")
                    ring(8 * 128 * HID).dma_start(
                        out=t, in_=w1s_d[:, 8 * i:8 * (i + 1), :])
                    w1s_ts.append(t)
                    t = data.tile([128, 4, HID], bf, tag=f"w1q{i}",
                                  name=f"w1q{i}")
                    ring(8 * 128 * HID).dma_start(
                        out=t, in_=w1p_d[:, 4 * i:4 * (i + 1), :])
                    w1p_ts.append(t)

                def w1_slice(hc, e):
                    if e < 2:
                        return w1s_ts[hc // 4][:, (hc % 4) * 2 + e, :]
                    return w1p_ts[hc // 4][:, hc % 4, :]
            else:
                w1f_ts = []
                for i in range(4):
                    t = data.tile([128, 6, HID], bf, tag=f"w1f{i}",
                                  name=f"w1f{i}")
                    ring(12 * 128 * HID).dma_start(
                        out=t, in_=w1f_d[:, 6 * i:6 * (i + 1), :])
                    w1f_ts.append(t)

                def w1_slice(hc, e):
                    kc = hc * 3 + e
                    return w1f_ts[kc // 6][:, kc % 6, :]

            # ACT table loads for Identity/Lrelu, after the DMA gens
            jact = work.tile([128, 32], bf, tag="jact")
            nc.scalar.activation(jact, junk[:, 0:32],
                                 mybir.ActivationFunctionType.Identity,
                                 scale=2.0)
            nc.scalar.activation(jact, junk[:, 0:32],
                                 mybir.ActivationFunctionType.Lrelu,
                                 alpha=0.01)

            # --- mm1: x[24, 1024] += mask_chunk.T @ bert_chunk over chunks
            px0 = psx.tile([NMC, 512], f32, tag="px0")
            px1 = psx.tile([NMC, 512], f32, tag="px1")

            def mm1_chunk(buf, lc, first, last):
                m = buf[:, lc, 0:MW]
                nc.tensor.matmul(px0, m, buf[:, lc, MW:MW + 512],
                                 start=first, stop=last)
                nc.tensor.matmul(px1, m, buf[:, lc, MW + 512:CW],
                                 start=first, stop=last)

            pidx = 0
            for c in range(nch8):
                if pidx + 1 < len(e3_ts) and c >= e3_ts[pidx + 1][0]:
                    pidx += 1
                c0, t = e3_ts[pidx]
                mm1_chunk(t, c - c0, c == 0, False)
            for c in range(nchb):
                mm1_chunk(bf_t, c, False, c == nchb - 1)

            # --- x: scale by 1/len (fp32) + cast bf16, split ACT/DVE
            xsb = work.tile([NMC, H], bf, tag="xsb")
            nc.scalar.activation(xsb[:, 0:512], px0,
                                 mybir.ActivationFunctionType.Identity,
                                 scale=sfac_t)
            nc.vector.tensor_scalar_mul(xsb[:, 512:H], px1, sfac_t)

            # --- mm2 (hc-major), each hc's transpose immediately followed
            # by its 3 k-chunk matmuls: h[8, 512] = x @ (W1*bn_s) + bn_bias
            ident24 = cstA_t[0:NMC, 0:NMC]
            ones8 = cstB_t[0:1, 512:520]
            bnbr = cstB_t[0:1, 0:512]
            xT = work.tile([128, HC, NMC], bf, tag="xT")
            ph = psh.tile([BPC, HID], f32, tag="ph")
            nc.tensor.matmul(ph, ones8, bnbr, start=True, stop=False)
            for hc in range(HC):
                pT = pst.tile([128, NMC], bf, tag="pT")
                nc.tensor.transpose(pT, xsb[:, hc * 128:(hc + 1) * 128],
                                    ident24)
                nc.vector.tensor_copy(xT[:, hc, :], pT)
                for e in range(3):
                    nc.tensor.matmul(
                        ph, xT[:, hc, e * BPC:(e + 1) * BPC], w1_slice(hc, e),
                        start=False, stop=(hc == HC - 1 and e == 2),
                    )

            # --- LeakyReLU on ACT: y = lrelu(h), cast bf16
            y = work.tile([BPC, HID], bf, tag="y")
            nc.scalar.activation(y, ph, mybir.ActivationFunctionType.Lrelu,
                                 alpha=0.01)

            # --- mm3: out[3, 8] = W2.T @ y.T + b2 (batched transposes)
            ident8 = cstA_t[0:BPC, 0:BPC]
            b2t = cstB_t[0:1, 520:523]
            yT_ps = [pst.tile([128, BPC], bf, tag="pT", name=f"yTp{mc}")
                     for mc in range(4)]
            yT_sb = [work.tile([128, BPC], bf, tag=f"yTs{mc}", name=f"yTs{mc}")
                     for mc in range(4)]
            for mc in range(4):
                nc.tensor.transpose(
                    yT_ps[mc], y[:, mc * 128:(mc + 1) * 128], ident8)
            for mc in range(4):
                nc.vector.tensor_copy(yT_sb[mc], yT_ps[mc])
            oT = psx.tile([3, BPC], f32, tag="px0")
            for mc in range(4):
                nc.tensor.matmul(oT, cstA_t[:, 24 + 3 * mc:27 + 3 * mc],
                                 yT_sb[mc], start=(mc == 0), stop=False)
            nc.tensor.matmul(oT, b2t, ones8, start=False, stop=True)

            o_sb = work.tile([3, BPC], f32, tag="osb")
            nc.vector.tensor_copy(o_sb, oT)
            nc.sync.dma_start(out=out_d, in_=o_sb)

    nc.compile()
    return nc


def _pack_rows(rows, masks, nch, np_dt, part=128):
    """rows: [N, H] fp32, masks: [N, NMC] fp32 -> [part, nch, CW] np_dt,
    partition-major (packed position i -> (p=i%part, c=i//part))."""
    N = rows.shape[0]
    buf = np.zeros((nch * part, CW), dtype=np.float32)
    if N:
        buf[:N, :MW] = masks
        buf[:N, MW:] = rows
    return np.ascontiguousarray(
        buf.reshape(nch, part, CW).transpose(1, 0, 2)).astype(np_dt)


def _prep_core_inputs(bert, offs, w1_bufs, cstA, cstB, batch_idx,
                      nch8, nchb, bp, np_sdt):
    import ml_dtypes
    bf16 = ml_dtypes.bfloat16
    f8_rows, f8_masks = [], []
    b16_rows, b16_masks = [], []
    wsc = W1_SCALE if W1_SPLIT else 1.0
    sfac = np.ones((NMC, 1), dtype=np.float32)
    for slot, gb in enumerate(batch_idx):
        a0, a1, b0, b1_, p = (int(v) for v in offs[gb])
        spans = [(a0, a1, 0), (b0, b1_, 1)]
        long_spans = [s for s in spans if s[1] - s[0] + 1 >= T_FP8]
        short_spans = [s for s in spans if s[1] - s[0] + 1 < T_FP8]
        for (lo, hi, e) in spans:
            sfac[e * BPC + slot, 0] = 1.0 / ((hi - lo + 1) * wsc)
        if long_spans:
            lo = min(s[0] for s in long_spans)
            hi = max(s[1] for s in long_spans)
            pos = np.arange(lo, hi + 1)
            keep = np.zeros(len(pos), dtype=bool)
            m = np.zeros((len(pos), NMC), dtype=np.float32)
            for (s0, s1, e) in long_spans:
                sel = (pos >= s0) & (pos <= s1)
                keep |= sel
                m[sel, e * BPC + slot] = 1.0
            f8_rows.append(bert[gb, pos[keep]])
            f8_masks.append(m[keep])
        want = {}
        for (s0, s1, e) in short_spans:
            for r in range(s0, s1 + 1):
                want.setdefault(r, []).append(e)
        want.setdefault(p, []).append(2)
        if want:
            rs = sorted(want)
            m = np.zeros((len(rs), NMC), dtype=np.float32)
            for i, r in enumerate(rs):
                for e in want[r]:
                    m[i, e * BPC + slot] = 1.0
            b16_rows.append(bert[gb, rs])
            b16_masks.append(m)

    def cat(parts, w):
        return (np.concatenate(parts, axis=0) if parts
                else np.zeros((0, w), dtype=np.float32))

    in_map = {
        "e3buf": _pack_rows(cat(f8_rows, H), cat(f8_masks, NMC), nch8, np_sdt),
        "bfbuf": _pack_rows(cat(b16_rows, H), cat(b16_masks, NMC), nchb, bf16,
                            part=bp),
        "cstA": cstA,
        "cstB": cstB,
        "sfac": sfac,
    }
    in_map.update(w1_bufs)
    return in_map


def _row_counts(offs):
    """Per-batch (fp8 rows, bf16 rows) under the T_FP8 split."""
    n8 = np.zeros(B, dtype=np.int64)
    nb = np.zeros(B, dtype=np.int64)
    for gb in range(B):
        a0, a1, b0, b1_, p = (int(v) for v in offs[gb])
        spans = [(a0, a1), (b0, b1_)]
        longs = [s for s in spans if s[1] - s[0] + 1 >= T_FP8]
        shorts = [s for s in spans if s[1] - s[0] + 1 < T_FP8]
        if longs:
            lo = min(s[0] for s in longs)
            hi = max(s[1] for s in longs)
            keep = np.zeros(hi - lo + 1, dtype=bool)
            for (s0, s1) in longs:
                keep[s0 - lo:s1 - lo + 1] = True
            n8[gb] = keep.sum()
        rows = set()
        for (s0, s1) in shorts:
            rows.update(range(s0, s1 + 1))
        rows.add(p)
        nb[gb] = len(rows)
    return n8, nb


def kernel(bert_outputs, offsets, W1, b1, gamma, beta, running_mean,
           running_var, W2, b2):
    import ml_dtypes
    bf16 = ml_dtypes.bfloat16
    e3 = ml_dtypes.float8_e3m4

    bert = np.ascontiguousarray(np.asarray(bert_outputs, dtype=np.float32))
    offs = np.asarray(offsets).astype(np.int64)
    W1 = np.asarray(W1, dtype=np.float32)
    b1 = np.asarray(b1, dtype=np.float32)
    gamma = np.asarray(gamma, dtype=np.float32)
    beta = np.asarray(beta, dtype=np.float32)
    rm = np.asarray(running_mean, dtype=np.float32)
    rv = np.asarray(running_var, dtype=np.float32)
    W2 = np.asarray(W2, dtype=np.float32)
    b2 = np.asarray(b2, dtype=np.float32)

    # Fold BN eval stats: bn(xW1 + b1) = x(W1*s) + ((b1 - mean)*s + beta)
    s = gamma / np.sqrt(rv + EPS)
    bias = (b1 - rm) * s + beta
    W1s = W1 * s[None, :]
    w1ehc = W1s.reshape(3, 8, 128, HID)  # [e, hc, p, n]
    if W1_SPLIT:
        w1_bufs = {
            # [p, hc*2+e, n] for span embeddings e in {0,1}
            "w1S": np.ascontiguousarray(
                w1ehc[:2].transpose(2, 1, 0, 3).reshape(128, 16, HID)
                * W1_SCALE).astype(e3),
            # [p, hc, n] for the pron embedding
            "w1P": np.ascontiguousarray(
                w1ehc[2].transpose(1, 0, 2)).astype(bf16),
        }
    else:
        # [p, hc*3+e, n]
        w1_bufs = {"w1F": np.ascontiguousarray(
            w1ehc.transpose(2, 1, 0, 3).reshape(128, 24, HID)).astype(bf16)}

    cstA = np.zeros((128, 36), dtype=np.float32)
    cstA[:NMC, :NMC] = np.eye(NMC)
    cstA[:, 24:36] = W2.reshape(4, 128, 3).transpose(1, 0, 2).reshape(128, 12)
    cstA = cstA.astype(bf16)
    cstB = np.zeros((1, 523), dtype=np.float32)
    cstB[0, 0:512] = bias
    cstB[0, 512:520] = 1.0
    cstB[0, 520:523] = b2
    cstB = cstB.astype(bf16)

    # Greedy-balance batches across cores by shipped bytes (fp8 row = 1048B,
    # bf16 row = 2096B), capped at BPC batches per core
    n8, nb = _row_counts(offs)
    cost = n8 + 2 * nb
    order = np.argsort(-cost, kind="stable")
    core_rows8 = np.zeros(NCORES, dtype=np.int64)
    core_rowsb = np.zeros(NCORES, dtype=np.int64)
    core_batches = [[] for _ in range(NCORES)]
    for gb in order:
        load = core_rows8 + 2 * core_rowsb
        load[np.array([len(cb) >= BPC for cb in core_batches])] = 1 << 40
        c = int(np.argmin(load))
        core_batches[c].append(int(gb))
        core_rows8[c] += n8[gb]
        core_rowsb[c] += nb[gb]
    nch8 = max(1, int((core_rows8.max() + 127) // 128))
    maxb = int(core_rowsb.max())
    if maxb <= 96:
        bp = max(32, (maxb + 31) // 32 * 32)
        nchb = 1
    else:
        bp = 128
        nchb = (maxb + 127) // 128

    key = (nch8, nchb, bp)
    if key not in _PROGRAM_CACHE:
        _PROGRAM_CACHE[key] = _build_program(nch8, nchb, bp)
    nc = _PROGRAM_CACHE[key]

    np_sdt = _span_np_dt()
    in_maps = [
        _prep_core_inputs(bert, offs, w1_bufs, cstA, cstB, core_batches[c],
                          nch8, nchb, bp, np_sdt)
        for c in range(NCORES)
    ]

    from concourse import bass_utils
    kwargs = {}
    if TRACE:
        kwargs = {"trace": True, "trace_cores": list(range(NCORES))}
    res = bass_utils.run_bass_kernel_spmd(nc, in_maps,
                                          core_ids=list(range(NCORES)),
                                          **kwargs)
    global LAST_RESULT
    LAST_RESULT = res

    out = np.empty((B, 3), dtype=np.float32)
    for c in range(NCORES):
        out[core_batches[c]] = res.results[c]["out"].T
    return out

